# revision 20
# baseline (speedup 1.0000x reference)
# Dynamic convolution (CondConv-style) Trainium2 Bass kernel.
#
# Problem: x [16, 128, 128, 128]; per-sample attention over K=4 expert
# 3x3 conv kernels; per-sample aggregated conv + bias.
#
# Strategy: data-parallel over batch, 2 samples per core on 8 cores.
# Per core / per sample:
#   1. DMA x into SBUF as [C=128, 130, 130] with a zero halo (pad=1).
#   2. DVE reduce_sum over free dims -> pooled mean [C, 1].
#   3. Attention: two tiny matmuls + softmax over K=4 (free dim).
#   4. Expert mixing fused with transpose: for each of 9 taps,
#      agg_wT[ci, co] = sum_k att_k * W_k[co, ci, tap] via 4 accumulating
#      PE transpose-matmuls whose moving operand is att_k * I.
#   5. Conv: for each 4-row output chunk (512 cols), 9 PSUM-accumulated
#      matmuls rhs = shifted window of the padded x.
#   6. Drain: bias add (per-partition scalar) PSUM->SBUF, DMA to DRAM.
import os

import numpy as np

B, C, H, W = 16, 128, 128, 128
K, HID, KS = 4, 64, 3
TEMP = 30.0
N_CORES = 8
BPC = B // N_CORES  # samples per core
HP, WP = H + 2, W + 2  # padded spatial
ROWS_PER_CHUNK = 4
NCHUNK = H // ROWS_PER_CHUNK
TAPS = KS * KS

_cache = {}


def _build(conv_f32r: bool, repeat: int = 1):
    """Build + compile the Bass program (same program for all 8 cores)."""
    import concourse.bacc as bacc
    import concourse.mybir as mybir
    import concourse.tile as tile
    from concourse.masks import make_identity

    fp32 = mybir.dt.float32
    f32r = mybir.dt.float32r
    AF = mybir.ActivationFunctionType
    AX = mybir.AxisListType

    nc = bacc.Bacc(
        "TRN2",
        target_bir_lowering=False,
        debug=False,
        enable_asserts=False,
        num_devices=N_CORES,
    )

    x_d = nc.dram_tensor("x", (BPC, C, H, W), fp32, kind="ExternalInput").ap()
    w1_d = nc.dram_tensor("att_w1", (HID, C), fp32, kind="ExternalInput").ap()
    w2_d = nc.dram_tensor("att_w2", (K, HID), fp32, kind="ExternalInput").ap()
    wgt_d = nc.dram_tensor("weight", (K, C, C, KS, KS), fp32, kind="ExternalInput").ap()
    bias_d = nc.dram_tensor("bias", (K, C), fp32, kind="ExternalInput").ap()
    out_d = nc.dram_tensor("out", (BPC, C, H, W), fp32, kind="ExternalOutput").ap()

    wgt_flat = wgt_d.rearrange("k o i kh kw -> k o (i kh kw)")
    out_flat = out_d.rearrange("b c h w -> b c (h w)")

    # fp32r matmul operands must be written by a compute op that rounds to
    # fp32r; DMA alone does not qualify. So x goes HBM->SBUF contiguous,
    # then a DVE copy re-lays it into the padded tile with fp32r output.
    conv_dt = f32r if conv_f32r else fp32

    from contextlib import ExitStack

    with tile.TileContext(nc) as tc, ExitStack() as ctx:
        consts = ctx.enter_context(tc.tile_pool(name="consts", bufs=1))
        xpool = ctx.enter_context(tc.tile_pool(name="xpool", bufs=1))
        wpool = ctx.enter_context(tc.tile_pool(name="wpool", bufs=1))
        smalls = ctx.enter_context(tc.tile_pool(name="smalls", bufs=1))
        stage = ctx.enter_context(tc.tile_pool(name="stage", bufs=6))
        cpsum = ctx.enter_context(tc.tile_pool(name="cpsum", bufs=5, space="PSUM"))
        apsum = ctx.enter_context(tc.tile_pool(name="apsum", bufs=2, space="PSUM"))
        spsum = ctx.enter_context(tc.tile_pool(name="spsum", bufs=1, space="PSUM"))

        # ---- global constants ----
        ident = consts.tile([C, C], fp32, name="ident")
        make_identity(nc, ident)
        ones_row = consts.tile([1, C], fp32, name="ones_row")
        nc.vector.memset(ones_row, 1.0)
        zero_col = consts.tile([C, HP], fp32, name="zero_col")
        nc.vector.memset(zero_col, 0.0)

        w1T = consts.tile([C, HID], fp32, name="w1T")
        nc.sync.dma_start(out=w1T, in_=w1_d.rearrange("h c -> c h"))
        w2T = consts.tile([HID, K], fp32, name="w2T")
        nc.sync.dma_start(out=w2T, in_=w2_d.rearrange("k h -> h k"))
        bias_sb = consts.tile([K, C], fp32, name="bias_sb")
        nc.sync.dma_start(out=bias_sb, in_=bias_d)

        # bias transposed to [C(out), K] via PE transpose
        biasT_ps = spsum.tile([C, K], fp32, name="biasT_ps", tag="sps")
        nc.tensor.matmul(
            biasT_ps, bias_sb, ident[:K, :K], is_transpose=True, start=True, stop=True
        )
        biasT = consts.tile([C, K], fp32, name="biasT")
        nc.vector.tensor_copy(biasT, biasT_ps)

        # expert weight bank, [co, ci, tap] per expert (contiguous DMA).
        # Deferred behind the first sample's x load (not needed until
        # attention is done) so that load gets full HBM bandwidth.
        w_sb = []
        wbank_dmas = []
        for k in range(K):
            wk = wpool.tile([C, C, TAPS], fp32, name=f"w_sb{k}")
            wd = nc.sync.dma_start(
                out=wk, in_=wgt_flat[k].rearrange("o (i t) -> o i t", t=TAPS)
            )
            wbank_dmas.append(wd)
            w_sb.append(wk)

        from concourse.bass import _add_dep_helper

        xtmp_pool = ctx.enter_context(tc.tile_pool(name="xtmp", bufs=3))
        QROWS = 16  # x staging chunk height
        prev_load_dmas = None
        for b in [b for _ in range(repeat) for b in range(BPC)]:
            # ---- load x (contiguous DMA), re-lay into padded tile ----
            x_pad = xpool.tile([C, HP, WP], conv_dt, name=f"x_pad{b}")
            nc.vector.tensor_copy(x_pad[:, 0, :], zero_col)
            nc.vector.tensor_copy(x_pad[:, HP - 1, :], zero_col)
            nc.vector.tensor_copy(x_pad[:, :, 0], zero_col)
            nc.vector.tensor_copy(x_pad[:, :, WP - 1], zero_col)
            nq = H // QROWS
            partials = smalls.tile([C, nq], fp32, name=f"partials{b}")
            load_dmas = []
            for q in range(nq):
                xt = xtmp_pool.tile([C, QROWS, W], fp32, name="xt")
                ld = nc.sync.dma_start(
                    out=xt, in_=x_d[b, :, q * QROWS : (q + 1) * QROWS, :]
                )
                # stagger this sample's load behind the previous sample's so
                # the first load gets full HBM bandwidth (conv0 starts sooner)
                if prev_load_dmas is not None:
                    _add_dep_helper(ld.ins, prev_load_dmas[q].ins, reason="stagger x loads")
                elif wbank_dmas is not None:
                    for wd in wbank_dmas:
                        _add_dep_helper(wd.ins, ld.ins, reason="defer weight bank")
                load_dmas.append(ld)
                nc.vector.tensor_copy(
                    x_pad[:, 1 + q * QROWS : 1 + (q + 1) * QROWS, 1 : W + 1], xt
                )
                # per-chunk partial sum overlaps the remaining DMAs
                nc.vector.reduce_sum(
                    out=partials[:, q : q + 1],
                    in_=xt.rearrange("c h w -> c (h w)"),
                    axis=AX.X,
                )

            # ---- pooled mean ----
            psum_col = smalls.tile([C, 1], fp32, name=f"psum_col{b}")
            nc.vector.reduce_sum(out=psum_col, in_=partials, axis=AX.X)
            pooled = smalls.tile([C, 1], fp32, name=f"pooled{b}")
            nc.scalar.mul(pooled, psum_col, 1.0 / (H * W))

            # ---- attention MLP ----
            h_ps = spsum.tile([HID, 1], fp32, name=f"h_ps{b}", tag="sps")
            nc.tensor.matmul(h_ps, w1T, pooled, start=True, stop=True)
            h_sb = smalls.tile([HID, 1], fp32, name=f"h_sb{b}")
            nc.scalar.activation(h_sb, h_ps, AF.Relu)

            log_ps = spsum.tile([1, K], fp32, name=f"log_ps{b}", tag="sps")
            nc.tensor.matmul(log_ps, h_sb, w2T, start=True, stop=True)

            # softmax over free dim (K=4), temperature 30
            lmax = smalls.tile([1, 1], fp32, name=f"lmax{b}")
            nc.vector.reduce_max(out=lmax, in_=log_ps, axis=AX.X)
            negb = smalls.tile([1, 1], fp32, name=f"negb{b}")
            nc.scalar.mul(negb, lmax, -1.0 / TEMP)
            att_e = smalls.tile([1, K], fp32, name=f"att_e{b}")
            nc.scalar.activation(att_e, log_ps, AF.Exp, bias=negb, scale=1.0 / TEMP)
            esum = smalls.tile([1, 1], fp32, name=f"esum{b}")
            nc.vector.reduce_sum(out=esum, in_=att_e, axis=AX.X)
            rsum = smalls.tile([1, 1], fp32, name=f"rsum{b}")
            nc.vector.reciprocal(rsum, esum)
            att_row = smalls.tile([1, K], fp32, name=f"att_row{b}")
            nc.vector.tensor_scalar_mul(att_row, att_e, rsum)

            # broadcast att row to all 128 partitions via K=1 matmul
            attb_ps = spsum.tile([C, K], fp32, name=f"attb_ps{b}", tag="sps")
            nc.tensor.matmul(attb_ps, ones_row, att_row, start=True, stop=True)
            att_bc = smalls.tile([C, K], fp32, name=f"att_bc{b}")
            nc.vector.tensor_copy(att_bc, attb_ps)

            # aggregated bias [C, 1]
            btmp = smalls.tile([C, K], fp32, name=f"btmp{b}")
            nc.vector.tensor_mul(btmp, biasT, att_bc)
            aggb = smalls.tile([C, 1], fp32, name=f"aggb{b}")
            nc.vector.reduce_sum(out=aggb, in_=btmp, axis=AX.X)

            # ---- expert mixing fused with transpose ----
            # scaled identities att_k * I
            sids = []
            for k in range(K):
                sid = wpool.tile([C, C], fp32, name=f"sid{b}_{k}")
                nc.vector.tensor_scalar_mul(sid, ident, att_bc[:, k : k + 1])
                sids.append(sid)

            wT = []
            for s in range(TAPS):
                agg_ps = apsum.tile([C, C], fp32, name="agg_ps")
                for k in range(K):
                    nc.tensor.matmul(
                        agg_ps,
                        w_sb[k][:, :, s],
                        sids[k],
                        start=(k == 0),
                        stop=(k == K - 1),
                    )
                wt = wpool.tile([C, C], conv_dt, name=f"wT{b}_{s}")
                nc.vector.tensor_copy(wt, agg_ps)
                wT.append(wt)

            # ---- convolution ----
            for chunk in range(NCHUNK):
                h0 = chunk * ROWS_PER_CHUNK
                cps = cpsum.tile([C, ROWS_PER_CHUNK * W], fp32, name="cps")
                for s in range(TAPS):
                    dy, dx = s // KS, s % KS
                    rhs = x_pad[:, h0 + dy : h0 + dy + ROWS_PER_CHUNK, dx : dx + W]
                    nc.tensor.matmul(
                        cps,
                        wT[s],
                        rhs,
                        start=(s == 0),
                        stop=(s == TAPS - 1),
                    )
                og = stage.tile([C, ROWS_PER_CHUNK * W], fp32, name="og")
                # bias add on the (otherwise idle) scalar engine
                nc.scalar.activation(og, cps, AF.Identity, bias=aggb, scale=1.0)
                nc.sync.dma_start(
                    out=out_flat[b, :, h0 * W : (h0 + ROWS_PER_CHUNK) * W], in_=og
                )
            prev_load_dmas = load_dmas

    nc.compile()
    return nc


def _get_prog():
    conv_f32r = os.environ.get("KERNEL_CONV_DTYPE", "f32r") == "f32r"
    repeat = int(os.environ.get("KERNEL_REPEAT", "1"))
    key = (conv_f32r, repeat)
    if key not in _cache:
        _cache[key] = _build(conv_f32r, repeat)
    return _cache[key]


def kernel(x, att_w1, att_w2, weight, bias):
    from concourse.bass_utils import run_bass_kernel_spmd

    nc = _get_prog()
    in_maps = []
    for i in range(N_CORES):
        in_maps.append(
            {
                "x": np.ascontiguousarray(x[i * BPC : (i + 1) * BPC]),
                "att_w1": np.asarray(att_w1),
                "att_w2": np.asarray(att_w2),
                "weight": np.asarray(weight),
                "bias": np.asarray(bias),
            }
        )
    res = run_bass_kernel_spmd(nc, in_maps, list(range(N_CORES)))
    kernel.last_results = res
    return np.concatenate([r["out"] for r in res.results], axis=0)


# revision 28
# speedup vs baseline: 1.1904x; 1.1904x over previous
# Dynamic convolution (CondConv-style) Trainium2 Bass kernel.
#
# Problem: x [16, 128, 128, 128]; per-sample attention over K=4 expert
# 3x3 conv kernels; per-sample aggregated conv + bias.
#
# Strategy: data-parallel over batch, 2 samples per core on 8 cores.
# Per core:
#   - One-time: PE-transpose the K expert banks to [ci, tap, co] (hidden
#     inside the x-load window; PSUM->SBUF copies on the scalar engine).
#   - Per sample: contiguous-DMA x into staging chunks; one ACT op per
#     chunk both re-lays it into the zero-haloed [C, 130, 130] fp32r tile
#     (the compute op provides the fp32r rounding the matmul verifier
#     requires) and accumulates the chunk sum for the pooled mean; tiny
#     attention MLP + softmax (no max-subtraction: logits/30 are <<1, and
#     the exp and its sum fuse into one ACT op); DVE expert mixing (8
#     passes over [C, 1152]) -> per-sample conv weights; conv as 32
#     chunks x 9 PSUM-accumulated fp32r matmuls (N=512); bias-add drain
#     on the scalar engine; DMA out.
import os

import numpy as np

B, C, H, W = 16, 128, 128, 128
K, HID, KS = 4, 64, 3
TEMP = 30.0
N_CORES = 8
BPC = B // N_CORES  # samples per core
HP, WP = H + 2, W + 2  # padded spatial
ROWS_PER_CHUNK = 4
NCHUNK = H // ROWS_PER_CHUNK
TAPS = KS * KS

_cache = {}


def _build(conv_f32r: bool, repeat: int = 1):
    """Build + compile the Bass program (same program for all 8 cores)."""
    from contextlib import ExitStack

    import concourse.bacc as bacc
    import concourse.mybir as mybir
    import concourse.tile as tile
    from concourse.bass import _add_dep_helper
    from concourse.masks import make_identity

    fp32 = mybir.dt.float32
    f32r = mybir.dt.float32r
    AF = mybir.ActivationFunctionType
    AX = mybir.AxisListType

    nc = bacc.Bacc(
        "TRN2",
        target_bir_lowering=False,
        debug=False,
        enable_asserts=False,
        num_devices=N_CORES,
    )

    x_d = nc.dram_tensor("x", (BPC, C, H, W), fp32, kind="ExternalInput").ap()
    w1_d = nc.dram_tensor("att_w1", (HID, C), fp32, kind="ExternalInput").ap()
    w2_d = nc.dram_tensor("att_w2", (K, HID), fp32, kind="ExternalInput").ap()
    wgt_d = nc.dram_tensor("weight", (K, C, C, KS, KS), fp32, kind="ExternalInput").ap()
    bias_d = nc.dram_tensor("bias", (K, C), fp32, kind="ExternalInput").ap()
    out_d = nc.dram_tensor("out", (BPC, C, H, W), fp32, kind="ExternalOutput").ap()

    wgt_flat = wgt_d.rearrange("k o i kh kw -> k o (i kh kw)")
    out_flat = out_d.rearrange("b c h w -> b c (h w)")
    conv_dt = f32r if conv_f32r else fp32

    with tile.TileContext(nc) as tc, ExitStack() as ctx:
        consts = ctx.enter_context(tc.tile_pool(name="consts", bufs=1))
        xpool = ctx.enter_context(tc.tile_pool(name="xpool", bufs=1))
        smalls = ctx.enter_context(tc.tile_pool(name="smalls", bufs=1))
        stage = ctx.enter_context(tc.tile_pool(name="stage", bufs=4))
        xtmp_pool = ctx.enter_context(tc.tile_pool(name="xtmp", bufs=3))
        cpsum = ctx.enter_context(tc.tile_pool(name="cpsum", bufs=4, space="PSUM"))
        apsum = ctx.enter_context(tc.tile_pool(name="apsum", bufs=3, space="PSUM"))
        spsum = ctx.enter_context(tc.tile_pool(name="spsum", bufs=1, space="PSUM"))

        # ---- global constants ----
        ident = consts.tile([C, C], fp32, name="ident")
        make_identity(nc, ident)
        ones_row = consts.tile([1, C], fp32, name="ones_row")
        nc.vector.memset(ones_row, 1.0)
        zero_col = consts.tile([C, HP], fp32, name="zero_col")
        nc.vector.memset(zero_col, 0.0)

        # attention weights: contiguous DMA + PE transpose (a strided DMA
        # of 4-byte elements would be far slower). w1T also absorbs the
        # 1/(H*W) mean normalization.
        w1_sb = consts.tile([HID, C], fp32, name="w1_sb")
        nc.sync.dma_start(out=w1_sb, in_=w1_d)
        w1T_ps = spsum.tile([C, HID], fp32, name="w1T_ps", tag="sps")
        nc.tensor.transpose(w1T_ps, w1_sb, ident[:HID, :HID])
        w1T = consts.tile([C, HID], fp32, name="w1T")
        nc.scalar.mul(w1T, w1T_ps, 1.0 / (H * W))

        w2_sb = consts.tile([K, HID], fp32, name="w2_sb")
        nc.sync.dma_start(out=w2_sb, in_=w2_d)
        w2T_ps = spsum.tile([HID, K], fp32, name="w2T_ps", tag="sps")
        nc.tensor.transpose(w2T_ps, w2_sb, ident[:K, :K])
        w2T = consts.tile([HID, K], fp32, name="w2T")
        nc.scalar.copy(w2T, w2T_ps)

        bias_sb = consts.tile([K, C], fp32, name="bias_sb")
        nc.sync.dma_start(out=bias_sb, in_=bias_d)
        biasT_ps = spsum.tile([C, K], fp32, name="biasT_ps", tag="sps")
        nc.tensor.transpose(biasT_ps, bias_sb, ident[:K, :K])
        biasT = consts.tile([C, K], fp32, name="biasT")
        nc.scalar.copy(biasT, biasT_ps)

        # ---- one-time: transpose expert banks to [ci, tap, co] ----
        wTe_pool = ctx.enter_context(tc.tile_pool(name="wTe", bufs=1))
        wTe = [wTe_pool.tile([C, TAPS, C], fp32, name=f"wTe{k}") for k in range(K)]
        wbank_dmas = []
        with tc.tile_pool(name="wbank", bufs=1) as wbank:
            w_sb = []
            for k in range(K):
                wk = wbank.tile([C, C, TAPS], fp32, name=f"w_sb{k}")
                wd = nc.sync.dma_start(
                    out=wk, in_=wgt_flat[k].rearrange("o (i t) -> o i t", t=TAPS)
                )
                wbank_dmas.append(wd)
                w_sb.append(wk)
            for k in range(K):
                for s in range(TAPS):
                    tp = apsum.tile([C, C], fp32, name="tp")
                    nc.tensor.transpose(tp, w_sb[k][:, :, s], ident)
                    nc.vector.tensor_copy(wTe[k][:, s, :], tp)
        wTe_flat = [t.rearrange("c s o -> c (s o)") for t in wTe]

        # mix pool reuses the released wbank region (stack allocator)
        mix_pool = ctx.enter_context(tc.tile_pool(name="mix", bufs=1))
        wT_all = [
            mix_pool.tile([C, TAPS, C], conv_dt, name=f"wT{b}") for b in range(BPC)
        ]
        acc_t = mix_pool.tile([C, TAPS * C], fp32, name="acc_t")
        tmp_t = mix_pool.tile([C, TAPS * C], fp32, name="tmp_t")

        QROWS = 8  # x staging chunk height (1 MiB per DMA)
        nq = H // QROWS
        prev_load_dmas = None
        aggb_all = [None] * BPC

        for rep in range(repeat):
            x_pad_all = []
            for b in range(BPC):
                # ---- phase B: load x, pooled mean, attention ----
                x_pad = xpool.tile([C, HP, WP], conv_dt, name=f"x_pad{b}")
                x_pad_all.append(x_pad)
                nc.vector.tensor_copy(x_pad[:, 0, :], zero_col)
                nc.vector.tensor_copy(x_pad[:, HP - 1, :], zero_col)
                nc.vector.tensor_copy(x_pad[:, :, 0], zero_col)
                nc.vector.tensor_copy(x_pad[:, :, WP - 1], zero_col)

                partials = smalls.tile([C, nq], fp32, name=f"partials{b}")
                load_dmas = []
                for q in range(nq):
                    xt = xtmp_pool.tile([C, QROWS, W], fp32, name="xt")
                    ld = nc.sync.dma_start(
                        out=xt, in_=x_d[b, :, q * QROWS : (q + 1) * QROWS, :]
                    )
                    # stagger: this sample's load chunk q waits for the
                    # previous sample's chunk q so the first load gets full
                    # HBM bandwidth and its conv starts sooner
                    if prev_load_dmas is not None:
                        _add_dep_helper(
                            ld.ins, prev_load_dmas[q].ins, reason="stagger x loads"
                        )
                    elif q == nq // 2:
                        # weight bank rides behind the first half of x0
                        for wd in wbank_dmas:
                            _add_dep_helper(wd.ins, ld.ins, reason="defer wbank")
                    load_dmas.append(ld)
                    # one ACT op: re-lay chunk into padded tile (fp32r
                    # rounding) AND accumulate its sum for the pooled mean
                    nc.scalar.activation(
                        x_pad[:, 1 + q * QROWS : 1 + (q + 1) * QROWS, 1 : W + 1],
                        xt,
                        AF.Copy,
                        accum_out=partials[:, q : q + 1],
                    )
                prev_load_dmas = load_dmas

                pooled = smalls.tile([C, 1], fp32, name=f"pooled{b}")
                nc.vector.reduce_sum(out=pooled, in_=partials, axis=AX.X)

                h_ps = spsum.tile([HID, 1], fp32, name=f"h_ps{b}", tag="sps")
                nc.tensor.matmul(h_ps, w1T, pooled, start=True, stop=True)
                h_sb = smalls.tile([HID, 1], fp32, name=f"h_sb{b}")
                nc.scalar.activation(h_sb, h_ps, AF.Relu)

                log_ps = spsum.tile([1, K], fp32, name=f"log_ps{b}", tag="sps")
                nc.tensor.matmul(log_ps, h_sb, w2T, start=True, stop=True)

                # softmax: logits/TEMP are tiny (pooled means of unit
                # gaussians), so skip the max-subtraction; exp + sum in one op
                att_e = smalls.tile([1, K], fp32, name=f"att_e{b}")
                esum = smalls.tile([1, 1], fp32, name=f"esum{b}")
                nc.scalar.activation(
                    att_e, log_ps, AF.Exp, scale=1.0 / TEMP, accum_out=esum
                )
                rsum = smalls.tile([1, 1], fp32, name=f"rsum{b}")
                nc.vector.reciprocal(rsum, esum)
                att_row = smalls.tile([1, K], fp32, name=f"att_row{b}")
                nc.vector.tensor_scalar_mul(att_row, att_e, rsum)

                # broadcast normalized att to all partitions via K=1 matmul
                attb_ps = spsum.tile([C, K], fp32, name=f"attb_ps{b}", tag="sps")
                nc.tensor.matmul(attb_ps, ones_row, att_row, start=True, stop=True)
                att_bc = smalls.tile([C, K], fp32, name=f"att_bc{b}")
                nc.vector.tensor_copy(att_bc, attb_ps)

                # aggregated bias [C, 1]
                btmp = smalls.tile([C, K], fp32, name=f"btmp{b}")
                nc.vector.tensor_mul(btmp, biasT, att_bc)
                aggb = smalls.tile([C, 1], fp32, name=f"aggb{b}")
                nc.vector.reduce_sum(out=aggb, in_=btmp, axis=AX.X)
                aggb_all[b] = aggb

                # ---- phase D: expert mixing on DVE -> wT[b] ----
                wT_f = wT_all[b].rearrange("c s o -> c (s o)")
                nc.vector.tensor_scalar_mul(acc_t, wTe_flat[0], att_bc[:, 0:1])
                for k in (1, 2):
                    nc.vector.tensor_scalar_mul(
                        tmp_t, wTe_flat[k], att_bc[:, k : k + 1]
                    )
                    nc.vector.tensor_add(acc_t, acc_t, tmp_t)
                nc.vector.tensor_scalar_mul(tmp_t, wTe_flat[3], att_bc[:, 3:4])
                nc.vector.tensor_add(wT_f, acc_t, tmp_t)

            # ---- phase E: convolution ----
            for b in range(BPC):
                for chunk in range(NCHUNK):
                    h0 = chunk * ROWS_PER_CHUNK
                    cps = cpsum.tile([C, ROWS_PER_CHUNK * W], fp32, name="cps")
                    for s in range(TAPS):
                        dy, dx = s // KS, s % KS
                        rhs = x_pad_all[b][
                            :, h0 + dy : h0 + dy + ROWS_PER_CHUNK, dx : dx + W
                        ]
                        nc.tensor.matmul(
                            cps,
                            wT_all[b][:, s, :],
                            rhs,
                            start=(s == 0),
                            stop=(s == TAPS - 1),
                        )
                    og = stage.tile([C, ROWS_PER_CHUNK * W], fp32, name="og")
                    nc.scalar.activation(
                        og, cps, AF.Identity, bias=aggb_all[b], scale=1.0
                    )
                    nc.sync.dma_start(
                        out=out_flat[b, :, h0 * W : (h0 + ROWS_PER_CHUNK) * W], in_=og
                    )

    nc.compile()
    return nc


def _get_prog():
    conv_f32r = os.environ.get("KERNEL_CONV_DTYPE", "f32r") == "f32r"
    repeat = int(os.environ.get("KERNEL_REPEAT", "1"))
    key = (conv_f32r, repeat)
    if key not in _cache:
        _cache[key] = _build(conv_f32r, repeat)
    return _cache[key]


def kernel(x, att_w1, att_w2, weight, bias):
    from concourse.bass_utils import run_bass_kernel_spmd

    nc = _get_prog()
    in_maps = []
    for i in range(N_CORES):
        in_maps.append(
            {
                "x": np.ascontiguousarray(x[i * BPC : (i + 1) * BPC]),
                "att_w1": np.asarray(att_w1),
                "att_w2": np.asarray(att_w2),
                "weight": np.asarray(weight),
                "bias": np.asarray(bias),
            }
        )
    res = run_bass_kernel_spmd(nc, in_maps, list(range(N_CORES)))
    kernel.last_results = res
    return np.concatenate([r["out"] for r in res.results], axis=0)
